# revision 28
# baseline (speedup 1.0000x reference)
"""Trainium2 Bass kernel for nn_CausalSelfAttention_16003048145608.

Problem (see reference semantics): B=4, T=1024, C=2048, H=16 heads, HD=128.
  qkv = x @ W_attn ; split q,k,v ; input-dependent RoPE positions t derived
  from a histogram of token_index over ALL batch rows + per-row gather/cumsum;
  RoPE(q,k) with per-token angle t; q[...,-1]=1, k[...,-1]=cumulative_scores;
  v *= exp(cumulative_scores); causal+padding softmax(q k^T/sqrt(HD)) @ v;
  out = y @ W_proj.

Sharding (8 cores): batch(4) x head-group(2 groups of 8 heads).
  Core c handles batch b=c//2, heads hg=c%2 (8 heads).  QKV is a column slice
  of W_attn (tensor parallel), attention is per-head, proj is a row slice of
  W_proj producing a partial (1024, 2048) output; host sums the 2 partials
  per batch (the only cross-core reduction).

On-core layouts (partition dim first):
  xT      (C=2048 -> 16 k-tiles of 128, T=1024)   bf16, resident
  qT/kT   (128 = head dim d, 8 heads * 1024 tok)  bf16  ("q_all"/"k_all")
  v       (128 = token tile, 8 tok-tiles * 1024 vcol) bf16 ("v_all"),
          computed token-major directly (x as stationary operand) so the
          attention O = P^T V matmul needs no transposes at all.
  scores  computed TRANSPOSED: sT(k, q) = lhsT(kT-tile).T @ qT -> softmax
          denominators via a ones(128,128) matmul (partition reduction on PE).
All matmuls bf16 with fp32 PSUM accumulation; positions/trig in fp32.
Padding is folded into the k last-channel override (cs + (pad-1)*1e9) so the
masked columns exp() to exactly 0 with no per-element mask work.
"""

import math
import numpy as np
import ml_dtypes

import concourse.bass as bass
import concourse.bacc as bacc
import concourse.tile as tile
from concourse import mybir
from concourse.bass_utils import run_bass_kernel_spmd

F32 = mybir.dt.float32
BF16 = mybir.dt.bfloat16
I32 = mybir.dt.int32
ALU = mybir.AluOpType
ACTF = mybir.ActivationFunctionType

B, T, C, H, HD = 4, 1024, 2048, 16, 128
NHC = 8                 # heads per core
NCORES = 8
KT = C // 128           # 16 contraction tiles for qkv
QTILES = T // 128       # 8
SCALE = 1.0 / math.sqrt(HD)
TWO_PI = 2.0 * math.pi
LN1E4 = math.log(10000.0)


DEBUG = False
REPEAT = 1   # emit the body N times (for slope-based HW timing)


def _emit(nc):
    # ---- DRAM I/O (per-core shapes; host feeds per-core slices) ----
    xT = nc.dram_tensor("xT", [C, T], BF16, kind="ExternalInput").ap()
    w_qk = nc.dram_tensor("w_qk", [C, 2 * NHC * HD], BF16, kind="ExternalInput").ap()
    w_v = nc.dram_tensor("w_v", [C, NHC * HD], BF16, kind="ExternalInput").ap()
    w_p = nc.dram_tensor("w_p", [NHC * HD, C], BF16, kind="ExternalInput").ap()
    tok_all = nc.dram_tensor("tok_all", [B * T], I32, kind="ExternalInput").ap()
    tok_own = nc.dram_tensor("tok_own", [T], I32, kind="ExternalInput").ap()
    cs = nc.dram_tensor("cs", [T], F32, kind="ExternalInput").ap()
    pad = nc.dram_tensor("pad", [T], F32, kind="ExternalInput").ap()
    out = nc.dram_tensor("out", [T, C], F32, kind="ExternalOutput").ap()

    dbg = None
    if DEBUG:
        dbg = {
            "dbg_rows": nc.dram_tensor("dbg_rows", [8, 1024], F32,
                                       kind="ExternalOutput").ap(),
            "dbg_trig": nc.dram_tensor("dbg_trig", [5, 128, 1024], F32,
                                       kind="ExternalOutput").ap(),
            "dbg_q": nc.dram_tensor("dbg_q", [128, 8192], BF16,
                                    kind="ExternalOutput").ap(),
            "dbg_k": nc.dram_tensor("dbg_k", [128, 8192], BF16,
                                    kind="ExternalOutput").ap(),
            "dbg_v": nc.dram_tensor("dbg_v", [128, 8192], BF16,
                                    kind="ExternalOutput").ap(),
            "dbg_y": nc.dram_tensor("dbg_y", [128, 8192], BF16,
                                    kind="ExternalOutput").ap(),
            "dbg_p": nc.dram_tensor("dbg_p", [128, 512], BF16,
                                    kind="ExternalOutput").ap(),
            "dbg_z": nc.dram_tensor("dbg_z", [128, 512], F32,
                                    kind="ExternalOutput").ap(),
        }

    with tile.TileContext(nc) as tc:
        for _ in range(REPEAT):
            _body(nc, tc, xT, w_qk, w_v, w_p, tok_all, tok_own, cs, pad, out, dbg)
    return nc


def _positions(nc, tc, persist, ps, tok_all, tok_own, cs, pad,
               cos_bf, sin_bf, evs_cols, klr_bf, dbg=None):
    """Input-dependent RoPE positions -> cos_bf / sin_bf (sin rows 0:64 are
    negated), plus evs_cols = exp(cs) columns and the k-last-channel row."""
    with tc.tile_pool(name="posp", bufs=1) as pp, \
         tc.tile_pool(name="ohp", bufs=3) as ohp:
        # scalar rows packed into partitions of one tile (1 row each)
        # scalar rows: the HW verifier requires TensorTensor/STT/scan SBUF
        # operands to share the SAME start partition, so every row lives at
        # partition 0 of its own tile.
        def mkrow(tag):
            r = pp.tile([1, 1024], F32, tag=tag, name=tag)
            return r

        R_CS, R_PAD, R_KLT = mkrow("r_cs"), mkrow("r_pad"), mkrow("r_klt")
        R_CNT, R_RCP, R_IDX = mkrow("r_cnt"), mkrow("r_rcp"), mkrow("r_idx")
        R_R, R_Z, R_T = mkrow("r_r"), mkrow("r_z"), mkrow("r_t")
        nc.vector.memset(R_Z[:], 0.0)

        def row(r):
            return r[:]

        cs_cols = pp.tile([128, QTILES], F32, tag="cs_cols")
        nc.sync.dma_start(cs_cols[:], cs.rearrange("(c p) -> p c", p=128))
        nc.scalar.activation(evs_cols[:], cs_cols[:], ACTF.Exp)

        nc.sync.dma_start(row(R_CS), cs.rearrange("(a t) -> a t", a=1))
        nc.sync.dma_start(row(R_PAD), pad.rearrange("(a t) -> a t", a=1))
        # k-last-channel override row: cs + (pad-1)*1e9  (padding -> -1e9)
        nc.vector.tensor_scalar(row(R_KLT), row(R_PAD), 1e9, -1e9, ALU.mult, ALU.add)
        nc.vector.tensor_tensor(klr_bf[:], row(R_KLT), row(R_CS), ALU.add)

        # ---------- iotas ----------
        iota_i32 = pp.tile([128, 1024], I32, tag="iota_i32")
        nc.gpsimd.iota(iota_i32[:], [[1, 1024]], base=0, channel_multiplier=0)
        iota_f32 = pp.tile([128, 1024], F32, tag="iota_f32")
        nc.vector.tensor_copy(iota_f32[:], iota_i32[:])

        cols = pp.tile([128, 16], F32, tag="cols")  # small per-partition columns
        C_IOTA, C_GE, C_RM, C_EARG, C_INVF = range(5)

        def col(i):
            return cols[:, i : i + 1]

        iotap_i32 = pp.tile([128, 1], I32, tag="iotap_i32")
        nc.gpsimd.iota(iotap_i32[:], [[1, 1]], base=0, channel_multiplier=1)
        nc.vector.tensor_copy(col(C_IOTA), iotap_i32[:])
        bin_cols = pp.tile([128, 8], F32, tag="bin_cols")
        for j in range(8):
            nc.vector.tensor_scalar(bin_cols[:, j : j + 1], col(C_IOTA),
                                    float(128 * j), None, ALU.add)

        # inv_freq column (128,1): rows r -> 10000^(-2*(r%64)/128)
        nc.vector.tensor_scalar(col(C_GE), col(C_IOTA), 64.0, None, ALU.is_ge)
        nc.vector.scalar_tensor_tensor(col(C_RM), col(C_GE), -64.0, col(C_IOTA),
                                       ALU.mult, ALU.add)
        nc.vector.tensor_scalar(col(C_EARG), col(C_RM), -2.0 * LN1E4 / 128.0,
                                None, ALU.mult)
        nc.scalar.activation(col(C_INVF), col(C_EARG), ACTF.Exp)

        # ---------- histogram of tok_all (counts over ALL batch rows) ----------
        tok_cols_i32 = pp.tile([128, 32], I32, tag="tok_cols_i32")
        nc.sync.dma_start(tok_cols_i32[:], tok_all.rearrange("(c p) -> p c", p=128))
        tok_cols = pp.tile([128, 32], F32, tag="tok_cols")
        nc.vector.tensor_copy(tok_cols[:], tok_cols_i32[:])
        ones_col = pp.tile([128, 1], BF16, tag="ones_col")
        nc.vector.memset(ones_col[:], 1.0)
        ones_rows = pp.tile([128, 128], F32, tag="ones_rows")
        nc.vector.memset(ones_rows[:], 1.0)

        counts_ps = [ps.tile([1, 512], F32, tag="att", bufs=4, name="counts_ps")
                     for _ in range(2)]
        for c in range(32):
            oh = ohp.tile([128, 1024], BF16, tag="oh")
            nc.vector.tensor_scalar(oh[:], iota_f32[:], tok_cols[:, c : c + 1],
                                    None, ALU.is_equal)
            for hf in range(2):
                nc.tensor.matmul(counts_ps[hf][:], ones_col[:],
                                 oh[:, hf * 512 : (hf + 1) * 512],
                                 start=(c == 0), stop=(c == 31))
        for hf in range(2):
            nc.vector.tensor_scalar(row(R_CNT)[:, hf * 512 : (hf + 1) * 512],
                                    counts_ps[hf][:], 1e-10, None, ALU.add)
        nc.vector.reciprocal_approx_fast(row(R_RCP), row(R_CNT))
        # zero-count bins are never gathered; clamp their huge recips to 0 so
        # any numerical dust in the gather matmul cannot be amplified by 1e10
        R_M = mkrow("r_m")
        nc.vector.tensor_scalar(row(R_M), row(R_CNT), 0.5, None, ALU.is_ge)
        R_RCPC = mkrow("r_rcpc")
        nc.vector.tensor_tensor(row(R_RCPC), row(R_RCP), row(R_M), ALU.mult)
        # row -> column layout via a DRAM bounce (SBUF->SBUF transposing DMA
        # miscopies on HW; DRAM->SBUF partition-major loads are the proven path)
        rcp_dram = nc.dram_tensor(f"scratch_rcp_{nc.next_id()}", [1024], F32).ap()
        nc.sync.dma_start(rcp_dram.rearrange("(a t) -> a t", a=1), row(R_RCPC))
        recip_cols = pp.tile([128, 8], F32, tag="recip_cols")
        nc.sync.dma_start(recip_cols[:], rcp_dram.rearrange("(c p) -> p c", p=128))

        # ---------- own-batch gather r[t] = recip[idx[t]] + cumsum -> t ----------
        idxr_i32 = pp.tile([1, T], I32, tag="idxr_i32")
        nc.sync.dma_start(idxr_i32[:], tok_own.rearrange("(a t) -> a t", a=1))
        nc.vector.tensor_copy(row(R_IDX), idxr_i32[:])

        idx_bcast = pp.tile([128, T], F32, tag="idx_bcast")
        for hf in range(2):
            ib_ps = ps.tile([128, 512], F32, tag="att", bufs=4, name="ib_ps")
            nc.tensor.matmul(ib_ps[:], ones_rows[0:1, :],
                             R_IDX[0:1, hf * 512 : (hf + 1) * 512],
                             start=True, stop=True)
            nc.vector.tensor_copy(idx_bcast[:, hf * 512 : (hf + 1) * 512], ib_ps[:])

        r_ps = [ps.tile([1, 512], F32, tag="att", bufs=4, name="r_ps")
                for _ in range(2)]
        for j in range(8):
            oht = ohp.tile([128, 1024], F32, tag="oht")
            nc.vector.tensor_scalar(oht[:], idx_bcast[:], bin_cols[:, j : j + 1],
                                    None, ALU.is_equal)
            for hf in range(2):
                nc.tensor.matmul(r_ps[hf][:], recip_cols[:, j : j + 1],
                                 oht[:, hf * 512 : (hf + 1) * 512],
                                 start=(j == 0), stop=(j == 7))
        for hf in range(2):
            nc.vector.tensor_copy(row(R_R)[:, hf * 512 : (hf + 1) * 512], r_ps[hf][:])
        nc.vector.tensor_tensor_scan(row(R_T), row(R_R), row(R_Z), 0.0,
                                     ALU.add, ALU.add)

        # ---------- freqs + sin/cos (fp32, range-reduced) ----------
        freqs = pp.tile([128, T], F32, tag="freqs")
        for hf in range(2):
            tb_ps = ps.tile([128, 512], F32, tag="att", bufs=4, name="tb_ps")
            nc.tensor.matmul(tb_ps[:], ones_rows[0:1, :],
                             R_T[0:1, hf * 512 : (hf + 1) * 512],
                             start=True, stop=True)
            nc.vector.tensor_scalar(freqs[:, hf * 512 : (hf + 1) * 512], tb_ps[:],
                                    col(C_INVF), None, ALU.mult)
        # f_red = freqs - 2*pi*int(freqs/(2*pi))  (freqs >= 0)
        scr_a = pp.tile([128, T], F32, tag="scr_a")
        scr_i = pp.tile([128, T], I32, tag="scr_i")
        scr_b = pp.tile([128, T], F32, tag="scr_b")
        scr_c = pp.tile([128, T], F32, tag="scr_c")
        nc.vector.tensor_scalar(scr_a[:], freqs[:], 1.0 / TWO_PI, None, ALU.mult)
        nc.vector.tensor_copy(scr_i[:], scr_a[:])
        nc.vector.tensor_copy(scr_b[:], scr_i[:])
        nc.vector.scalar_tensor_tensor(scr_c[:], scr_b[:], -TWO_PI, freqs[:],
                                       ALU.mult, ALU.add)          # f_red
        nc.vector.add_range_wrap(scr_a[:], scr_c[:], 0.0, math.pi, TWO_PI)
        sin_f32 = pp.tile([128, T], F32, tag="sin_f32")
        nc.scalar.activation(sin_f32[:], scr_a[:], ACTF.Sin)
        nc.vector.add_range_wrap(scr_b[:], scr_c[:], math.pi / 2.0, math.pi, TWO_PI)
        cos_f32 = pp.tile([128, T], F32, tag="cos_f32")
        nc.scalar.activation(cos_f32[:], scr_b[:], ACTF.Sin)
        nc.vector.tensor_copy(cos_bf[:], cos_f32[:])
        # sin rows 0:64 negated (rotate_half sign folded in)
        nc.vector.tensor_scalar(sin_bf[0:64, :], sin_f32[0:64, :], -1.0,
                                None, ALU.mult)
        nc.vector.tensor_copy(sin_bf[64:128, :], sin_f32[64:128, :])
        if dbg is not None:
            for i, r in enumerate([R_CNT, R_RCP, R_IDX, R_R, R_T]):
                nc.sync.dma_start(dbg["dbg_rows"][i : i + 1, :], r[:])
            nc.sync.dma_start(dbg["dbg_trig"][0], scr_c[:])   # f_red
            nc.sync.dma_start(dbg["dbg_trig"][1], scr_a[:])   # sin_in
            nc.sync.dma_start(dbg["dbg_trig"][2], scr_b[:])   # cos_in
            nc.sync.dma_start(dbg["dbg_trig"][3], sin_f32[:])
            nc.sync.dma_start(dbg["dbg_trig"][4], cos_f32[:])


def _body(nc, tc, xT, w_qk, w_v, w_p, tok_all, tok_own, cs, pad, out, dbg=None):
    from contextlib import ExitStack

    with ExitStack() as ctx:
        persist = ctx.enter_context(tc.tile_pool(name="persist", bufs=1))
        ps = ctx.enter_context(tc.tile_pool(name="ps", bufs=3, space="PSUM"))
        pt_pool = ctx.enter_context(tc.tile_pool(name="pt", bufs=12))

        # ---------- persistent tiles ----------
        q_all = persist.tile([128, NHC * T], BF16, tag="q_all")
        k_all = persist.tile([128, NHC * T], BF16, tag="k_all")
        v_all = persist.tile([128, QTILES * NHC * HD], BF16, tag="v_all")
        y_big = persist.tile([128, NHC * T], BF16, tag="y_big")
        cos_bf = persist.tile([128, T], BF16, tag="cos_bf")
        sin_bf = persist.tile([128, T], BF16, tag="sin_bf")
        ones_sq = persist.tile([128, 128], BF16, tag="ones_sq")
        evs_cols = persist.tile([128, QTILES], F32, tag="evs_cols")
        klr_bf = persist.tile([1, T], BF16, tag="klr_bf")
        nc.vector.memset(ones_sq[:], 1.0)

        # ---------- positions / trig / small rows (scoped; SBUF released) ----
        _positions(nc, tc, persist, ps, tok_all, tok_own, cs, pad,
                   cos_bf, sin_bf, evs_cols, klr_bf, dbg)

        # ========== QKV + RoPE + attention, pipelined per head ==========
        with tc.tile_pool(name="bigin", bufs=1) as bigin, \
             tc.tile_pool(name="wstream", bufs=4) as wst, \
             tc.tile_pool(name="rope", bufs=3) as rope_pool, \
             tc.tile_pool(name="attp", bufs=3) as attp:
            xT_sb = bigin.tile([128, KT * T], BF16, tag="xT_sb")
            for k in range(KT):
                nc.sync.dma_start(xT_sb[:, k * T : (k + 1) * T],
                                  xT[k * 128 : (k + 1) * 128, :])
            w_v_sb = bigin.tile([128, KT * NHC * HD], BF16, tag="w_v_sb")
            for k in range(KT):
                nc.sync.dma_start(w_v_sb[:, k * 1024 : (k + 1) * 1024],
                                  w_v[k * 128 : (k + 1) * 128, :])
            one_row = bigin.tile([1, T], BF16, tag="one_row")
            nc.vector.memset(one_row[:], 1.0)

            # v first: out(tok, vcol) = xT_tile.T @ w_v (token-major, evs-scaled)
            for mt in range(QTILES):
                vv_ps = [ps.tile([128, 512], F32, tag="mm", bufs=4, name="vv_ps")
                         for _ in range(2)]
                for k in range(KT):
                    for nc2 in range(2):
                        nc.tensor.matmul(
                            vv_ps[nc2][:],
                            xT_sb[:, k * T + mt * 128 : k * T + mt * 128 + 128],
                            w_v_sb[:, k * 1024 + nc2 * 512 : k * 1024 + nc2 * 512 + 512],
                            start=(k == 0), stop=(k == KT - 1))
                for nc2 in range(2):
                    nc.vector.tensor_scalar(
                        v_all[:, mt * 1024 + nc2 * 512 : mt * 1024 + nc2 * 512 + 512],
                        vv_ps[nc2][:], evs_cols[:, mt : mt + 1], None, ALU.mult)

            for h in range(NHC):
                # --- q,k matmuls for this head (w_qk host layout: [qh|kh] per head)
                qk_ps = [[ps.tile([128, 512], F32, tag="mm", bufs=4, name="qk_ps")
                          for _ in range(2)] for _ in range(2)]
                for k in range(KT):
                    wt = wst.tile([128, 256], BF16, tag="wt")
                    nc.sync.dma_start(wt[:], w_qk[k * 128 : (k + 1) * 128,
                                                  h * 256 : (h + 1) * 256])
                    for t2 in range(2):
                        for nc2 in range(2):
                            nc.tensor.matmul(
                                qk_ps[t2][nc2][:], wt[:, t2 * 128 : t2 * 128 + 128],
                                xT_sb[:, k * T + nc2 * 512 : k * T + nc2 * 512 + 512],
                                start=(k == 0), stop=(k == KT - 1))
                for t2, dst in ((0, q_all), (1, k_all)):
                    for nc2 in range(2):
                        nc.scalar.copy(
                            dst[:, h * T + nc2 * 512 : h * T + nc2 * 512 + 512],
                            qk_ps[t2][nc2][:])

                # --- RoPE on this head
                sl = slice(h * T, (h + 1) * T)
                for t_all in (q_all, k_all):
                    rot = rope_pool.tile([128, T], BF16, tag="rot", name="rot")
                    nc.sync.dma_start(rot[0:64, :], t_all[64:128, sl])
                    nc.sync.dma_start(rot[64:128, :], t_all[0:64, sl])
                    tmp = rope_pool.tile([128, T], BF16, tag="ropetmp", name="tmp")
                    nc.vector.tensor_tensor(tmp[:], t_all[:, sl], cos_bf[:], ALU.mult)
                    nc.vector.tensor_tensor(rot[:], rot[:], sin_bf[:], ALU.mult)
                    nc.vector.tensor_tensor(t_all[:, sl], tmp[:], rot[:], ALU.add)
                # last-rotary-channel overrides (engine ops can't address
                # partition 127 -> write via DMA)
                nc.sync.dma_start(q_all[127:128, sl], one_row[:])
                nc.sync.dma_start(k_all[127:128, sl], klr_bf[:])
                if dbg is not None and h == NHC - 1:
                    nc.sync.dma_start(dbg["dbg_q"][:], q_all[:])
                    nc.sync.dma_start(dbg["dbg_k"][:], k_all[:])
                    nc.sync.dma_start(dbg["dbg_v"][:], v_all[:])

                # --- attention for this head (transposed scores)
                qh = q_all[:, sl]
                for qc in range(2):
                    ktmax = (qc + 1) * 4
                    p_tiles = []
                    for kt in range(ktmax):
                        s_ps = ps.tile([128, 512], F32, tag="att", bufs=4,
                                       name="s_ps")
                        nc.tensor.matmul(
                            s_ps[:],
                            k_all[:, h * T + kt * 128 : h * T + kt * 128 + 128],
                            qh[:, qc * 512 : qc * 512 + 512],
                            start=True, stop=True)
                        p_sb = pt_pool.tile([128, 512], BF16, tag="p", name="p_sb")
                        nc.scalar.activation(p_sb[:], s_ps[:], ACTF.Exp, scale=SCALE)
                        if qc * 4 <= kt:  # diagonal-crossing tile: zero k > q
                            nc.gpsimd.affine_select(
                                p_sb[:], p_sb[:], [[1, 512]], ALU.is_ge, 0.0,
                                base=qc * 512 - kt * 128, channel_multiplier=-1)
                        if dbg is not None and h == 0 and qc == 0 and kt == 0:
                            nc.sync.dma_start(dbg["dbg_p"][:], p_sb[:])
                        p_tiles.append(p_sb)

                    z_ps = ps.tile([128, 512], F32, tag="att", bufs=4, name="z_ps")
                    for kt in range(ktmax):
                        nc.tensor.matmul(z_ps[:], ones_sq[:], p_tiles[kt][:],
                                         start=(kt == 0), stop=(kt == ktmax - 1))
                    z_sb = attp.tile([128, 512], F32, tag="z_sb")
                    nc.vector.tensor_copy(z_sb[:], z_ps[:])
                    if dbg is not None and h == 0 and qc == 0:
                        nc.sync.dma_start(dbg["dbg_z"][:], z_sb[:])
                    rz32 = attp.tile([128, 512], F32, tag="rz32")
                    nc.vector.reciprocal_approx_fast(rz32[:], z_sb[:])
                    rzb = attp.tile([128, 512], BF16, tag="rzb")
                    nc.vector.tensor_copy(rzb[:], rz32[:])

                    y_ps = ps.tile([128, 512], F32, tag="att", bufs=4, name="y_ps")
                    for kt in range(ktmax):
                        nc.tensor.matmul(
                            y_ps[:],
                            v_all[:, kt * 1024 + h * 128 : kt * 1024 + h * 128 + 128],
                            p_tiles[kt][:], start=(kt == 0), stop=(kt == ktmax - 1))
                    y_sb = attp.tile([128, 512], BF16, tag="y_sb")
                    nc.scalar.copy(y_sb[:], y_ps[:])
                    nc.vector.tensor_tensor(
                        y_big[:, h * T + qc * 512 : h * T + qc * 512 + 512],
                        y_sb[:], rzb[:], ALU.mult)

        if dbg is not None:
            nc.sync.dma_start(dbg["dbg_y"][:], y_big[:])

        # ================= output projection (partial) =================
        with tc.tile_pool(name="wpp", bufs=1) as wpp, \
             tc.tile_pool(name="outp", bufs=3) as outp:
            w_p_sb = wpp.tile([128, NHC * C], BF16, tag="w_p_sb")
            for h8 in range(NHC):
                nc.sync.dma_start(w_p_sb[:, h8 * C : (h8 + 1) * C],
                                  w_p[h8 * 128 : (h8 + 1) * 128, :])
            for qt in range(QTILES):
                o_ps4 = [ps.tile([128, 512], F32, tag="mm", bufs=4, name="o_ps")
                         for _ in range(4)]
                for h8 in range(NHC):
                    for n4 in range(4):
                        nc.tensor.matmul(
                            o_ps4[n4][:],
                            y_big[:, h8 * T + qt * 128 : h8 * T + qt * 128 + 128],
                            w_p_sb[:, h8 * C + n4 * 512 : h8 * C + n4 * 512 + 512],
                            start=(h8 == 0), stop=(h8 == NHC - 1))
                for n4 in range(4):
                    o_sb = outp.tile([128, 512], F32, tag="o_sb")
                    nc.scalar.copy(o_sb[:], o_ps4[n4][:])
                    nc.sync.dma_start(
                        out[qt * 128 : (qt + 1) * 128, n4 * 512 : (n4 + 1) * 512],
                        o_sb[:])


_NC_CACHE = None


def _get_nc():
    global _NC_CACHE
    if _NC_CACHE is None:
        nc = bacc.Bacc("TRN2", target_bir_lowering=False, debug=False,
                       num_devices=NCORES)
        _emit(nc)
        nc.compile()
        _NC_CACHE = nc
    return _NC_CACHE


def make_in_maps(x, cumulative_scores, token_index, padding_mask, W_attn, W_proj):
    bf = ml_dtypes.bfloat16
    x = np.asarray(x, np.float32)
    csf = np.asarray(cumulative_scores, np.float32)
    tok = np.asarray(token_index, np.int32)
    padf = np.asarray(padding_mask, np.float32)
    Wa = np.asarray(W_attn, np.float32)
    Wp = np.asarray(W_proj, np.float32)

    tok_all = np.ascontiguousarray(tok.reshape(B * T))
    in_maps = []
    for core in range(NCORES):
        b, hg = core // 2, core % 2
        cols = slice(hg * 1024, (hg + 1) * 1024)
        wq = Wa[:, 0 * C : 1 * C][:, cols]
        wk = Wa[:, 1 * C : 2 * C][:, cols]
        wv = Wa[:, 2 * C : 3 * C][:, cols]
        in_maps.append({
            "xT": np.ascontiguousarray(x[b].T).astype(bf),
            "w_qk": np.ascontiguousarray(np.concatenate(
                [np.concatenate([wq[:, hh * 128 : (hh + 1) * 128],
                                 wk[:, hh * 128 : (hh + 1) * 128]], axis=1)
                 for hh in range(NHC)], axis=1)).astype(bf),
            "w_v": np.ascontiguousarray(wv).astype(bf),
            "w_p": np.ascontiguousarray(Wp[hg * 1024 : (hg + 1) * 1024, :]).astype(bf),
            "tok_all": tok_all,
            "tok_own": np.ascontiguousarray(tok[b]),
            "cs": np.ascontiguousarray(csf[b]),
            "pad": np.ascontiguousarray(padf[b]),
        })
    return in_maps


def kernel(x, cumulative_scores, token_index, padding_mask, W_attn, W_proj):
    nc = _get_nc()
    in_maps = make_in_maps(x, cumulative_scores, token_index, padding_mask,
                           W_attn, W_proj)
    res = run_bass_kernel_spmd(nc, in_maps, list(range(NCORES)))
    outs = [res.results[c]["out"] for c in range(NCORES)]
    full = np.stack([outs[2 * b] + outs[2 * b + 1] for b in range(B)], axis=0)
    return full.astype(np.float32)


# revision 29
# speedup vs baseline: 1.3117x; 1.3117x over previous
"""Trainium2 Bass kernel for nn_CausalSelfAttention_16003048145608.

Problem (see reference semantics): B=4, T=1024, C=2048, H=16 heads, HD=128.
  qkv = x @ W_attn ; split q,k,v ; input-dependent RoPE positions t derived
  from a histogram of token_index over ALL batch rows + per-row gather/cumsum;
  RoPE(q,k) with per-token angle t; q[...,-1]=1, k[...,-1]=cumulative_scores;
  v *= exp(cumulative_scores); causal+padding softmax(q k^T/sqrt(HD)) @ v;
  out = y @ W_proj.

Sharding (8 cores): batch(4) x head-group(2 groups of 8 heads).
  Core c handles batch b=c//2, heads hg=c%2 (8 heads).  QKV is a column slice
  of W_attn (tensor parallel), attention is per-head, proj is a row slice of
  W_proj producing a partial (1024, 2048) output; host sums the 2 partials
  per batch (the only cross-core reduction).

On-core layouts (partition dim first):
  xT      (C=2048 -> 16 k-tiles of 128, T=1024)   bf16, resident
  qT/kT   (128 = head dim d, 8 heads * 1024 tok)  bf16  ("q_all"/"k_all")
  v       (128 = token tile, 8 tok-tiles * 1024 vcol) bf16 ("v_all"),
          computed token-major directly (x as stationary operand) so the
          attention O = P^T V matmul needs no transposes at all.
  scores  computed TRANSPOSED: sT(k, q) = lhsT(kT-tile).T @ qT -> softmax
          denominators via a ones(128,128) matmul (partition reduction on PE).
All matmuls bf16 with fp32 PSUM accumulation; positions/trig in fp32.
Padding is folded into the k last-channel override (cs + (pad-1)*1e9) so the
masked columns exp() to exactly 0 with no per-element mask work.
"""

import math
import numpy as np
import ml_dtypes

import concourse.bass as bass
import concourse.bacc as bacc
import concourse.tile as tile
from concourse import mybir
from concourse.bass_utils import run_bass_kernel_spmd

F32 = mybir.dt.float32
BF16 = mybir.dt.bfloat16
I32 = mybir.dt.int32
ALU = mybir.AluOpType
ACTF = mybir.ActivationFunctionType

B, T, C, H, HD = 4, 1024, 2048, 16, 128
NHC = 8                 # heads per core
NCORES = 8
KT = C // 128           # 16 contraction tiles for qkv
QTILES = T // 128       # 8
SCALE = 1.0 / math.sqrt(HD)
TWO_PI = 2.0 * math.pi
LN1E4 = math.log(10000.0)


DEBUG = False
REPEAT = 1   # emit the body N times (for slope-based HW timing)


def _emit(nc):
    # ---- DRAM I/O (per-core shapes; host feeds per-core slices) ----
    xT = nc.dram_tensor("xT", [C, T], BF16, kind="ExternalInput").ap()
    w_qk = nc.dram_tensor("w_qk", [C, 2 * NHC * HD], BF16, kind="ExternalInput").ap()
    w_v = nc.dram_tensor("w_v", [C, NHC * HD], BF16, kind="ExternalInput").ap()
    w_p = nc.dram_tensor("w_p", [NHC * HD, C], BF16, kind="ExternalInput").ap()
    tok_all = nc.dram_tensor("tok_all", [B * T], I32, kind="ExternalInput").ap()
    tok_own = nc.dram_tensor("tok_own", [T], I32, kind="ExternalInput").ap()
    cs = nc.dram_tensor("cs", [T], F32, kind="ExternalInput").ap()
    pad = nc.dram_tensor("pad", [T], F32, kind="ExternalInput").ap()
    out = nc.dram_tensor("out", [T, C], F32, kind="ExternalOutput").ap()

    dbg = None
    if DEBUG:
        dbg = {
            "dbg_rows": nc.dram_tensor("dbg_rows", [8, 1024], F32,
                                       kind="ExternalOutput").ap(),
            "dbg_trig": nc.dram_tensor("dbg_trig", [5, 128, 1024], F32,
                                       kind="ExternalOutput").ap(),
            "dbg_q": nc.dram_tensor("dbg_q", [128, 8192], BF16,
                                    kind="ExternalOutput").ap(),
            "dbg_k": nc.dram_tensor("dbg_k", [128, 8192], BF16,
                                    kind="ExternalOutput").ap(),
            "dbg_v": nc.dram_tensor("dbg_v", [128, 8192], BF16,
                                    kind="ExternalOutput").ap(),
            "dbg_y": nc.dram_tensor("dbg_y", [128, 8192], BF16,
                                    kind="ExternalOutput").ap(),
            "dbg_p": nc.dram_tensor("dbg_p", [128, 512], BF16,
                                    kind="ExternalOutput").ap(),
            "dbg_z": nc.dram_tensor("dbg_z", [128, 512], F32,
                                    kind="ExternalOutput").ap(),
        }

    with tile.TileContext(nc) as tc:
        for _ in range(REPEAT):
            _body(nc, tc, xT, w_qk, w_v, w_p, tok_all, tok_own, cs, pad, out, dbg)
    return nc


def _positions(nc, tc, persist, ps, tok_all, tok_own, cs, pad,
               cos_bf, sin_bf, evs_cols, klr_bf, dbg=None):
    """Input-dependent RoPE positions -> cos_bf / sin_bf (sin rows 0:64 are
    negated), plus evs_cols = exp(cs) columns and the k-last-channel row."""
    with tc.tile_pool(name="posp", bufs=1) as pp, \
         tc.tile_pool(name="ohp", bufs=3) as ohp:
        # scalar rows packed into partitions of one tile (1 row each)
        # scalar rows: the HW verifier requires TensorTensor/STT/scan SBUF
        # operands to share the SAME start partition, so every row lives at
        # partition 0 of its own tile.
        def mkrow(tag):
            r = pp.tile([1, 1024], F32, tag=tag, name=tag)
            return r

        R_CS, R_PAD, R_KLT = mkrow("r_cs"), mkrow("r_pad"), mkrow("r_klt")
        R_CNT, R_RCP, R_IDX = mkrow("r_cnt"), mkrow("r_rcp"), mkrow("r_idx")
        R_R, R_Z, R_T = mkrow("r_r"), mkrow("r_z"), mkrow("r_t")
        nc.vector.memset(R_Z[:], 0.0)

        def row(r):
            return r[:]

        cs_cols = pp.tile([128, QTILES], F32, tag="cs_cols")
        nc.sync.dma_start(cs_cols[:], cs.rearrange("(c p) -> p c", p=128))
        nc.scalar.activation(evs_cols[:], cs_cols[:], ACTF.Exp)

        nc.sync.dma_start(row(R_CS), cs.rearrange("(a t) -> a t", a=1))
        nc.sync.dma_start(row(R_PAD), pad.rearrange("(a t) -> a t", a=1))
        # k-last-channel override row: cs + (pad-1)*1e9  (padding -> -1e9)
        nc.vector.tensor_scalar(row(R_KLT), row(R_PAD), 1e9, -1e9, ALU.mult, ALU.add)
        nc.vector.tensor_tensor(klr_bf[:], row(R_KLT), row(R_CS), ALU.add)

        # ---------- iotas ----------
        iota_i32 = pp.tile([128, 1024], I32, tag="iota_i32")
        nc.gpsimd.iota(iota_i32[:], [[1, 1024]], base=0, channel_multiplier=0)
        iota_f32 = pp.tile([128, 1024], F32, tag="iota_f32")
        nc.vector.tensor_copy(iota_f32[:], iota_i32[:])

        cols = pp.tile([128, 16], F32, tag="cols")  # small per-partition columns
        C_IOTA, C_GE, C_RM, C_EARG, C_INVF = range(5)

        def col(i):
            return cols[:, i : i + 1]

        iotap_i32 = pp.tile([128, 1], I32, tag="iotap_i32")
        nc.gpsimd.iota(iotap_i32[:], [[1, 1]], base=0, channel_multiplier=1)
        nc.vector.tensor_copy(col(C_IOTA), iotap_i32[:])
        bin_cols = pp.tile([128, 8], F32, tag="bin_cols")
        for j in range(8):
            nc.vector.tensor_scalar(bin_cols[:, j : j + 1], col(C_IOTA),
                                    float(128 * j), None, ALU.add)

        # inv_freq column (128,1): rows r -> 10000^(-2*(r%64)/128)
        nc.vector.tensor_scalar(col(C_GE), col(C_IOTA), 64.0, None, ALU.is_ge)
        nc.vector.scalar_tensor_tensor(col(C_RM), col(C_GE), -64.0, col(C_IOTA),
                                       ALU.mult, ALU.add)
        nc.vector.tensor_scalar(col(C_EARG), col(C_RM), -2.0 * LN1E4 / 128.0,
                                None, ALU.mult)
        nc.scalar.activation(col(C_INVF), col(C_EARG), ACTF.Exp)

        # ---------- histogram of tok_all (counts over ALL batch rows) ----------
        tok_cols_i32 = pp.tile([128, 32], I32, tag="tok_cols_i32")
        nc.sync.dma_start(tok_cols_i32[:], tok_all.rearrange("(c p) -> p c", p=128))
        tok_cols = pp.tile([128, 32], F32, tag="tok_cols")
        nc.vector.tensor_copy(tok_cols[:], tok_cols_i32[:])
        ones_col = pp.tile([128, 1], BF16, tag="ones_col")
        nc.vector.memset(ones_col[:], 1.0)
        ones_rows = pp.tile([128, 128], F32, tag="ones_rows")
        nc.vector.memset(ones_rows[:], 1.0)

        counts_ps = [ps.tile([1, 512], F32, tag="att", bufs=4, name="counts_ps")
                     for _ in range(2)]
        for c in range(32):
            oh = ohp.tile([128, 1024], BF16, tag="oh")
            nc.vector.tensor_scalar(oh[:], iota_f32[:], tok_cols[:, c : c + 1],
                                    None, ALU.is_equal)
            for hf in range(2):
                nc.tensor.matmul(counts_ps[hf][:], ones_col[:],
                                 oh[:, hf * 512 : (hf + 1) * 512],
                                 start=(c == 0), stop=(c == 31))
        for hf in range(2):
            nc.vector.tensor_scalar(row(R_CNT)[:, hf * 512 : (hf + 1) * 512],
                                    counts_ps[hf][:], 1e-10, None, ALU.add)
        nc.vector.reciprocal_approx_fast(row(R_RCP), row(R_CNT))
        # zero-count bins are never gathered; clamp their huge recips to 0 so
        # any numerical dust in the gather matmul cannot be amplified by 1e10
        R_M = mkrow("r_m")
        nc.vector.tensor_scalar(row(R_M), row(R_CNT), 0.5, None, ALU.is_ge)
        R_RCPC = mkrow("r_rcpc")
        nc.vector.tensor_tensor(row(R_RCPC), row(R_RCP), row(R_M), ALU.mult)
        # row -> column layout via a DRAM bounce (SBUF->SBUF transposing DMA
        # miscopies on HW; DRAM->SBUF partition-major loads are the proven path)
        rcp_dram = nc.dram_tensor(f"scratch_rcp_{nc.next_id()}", [1024], F32).ap()
        nc.sync.dma_start(rcp_dram.rearrange("(a t) -> a t", a=1), row(R_RCPC))
        recip_cols = pp.tile([128, 8], F32, tag="recip_cols")
        nc.sync.dma_start(recip_cols[:], rcp_dram.rearrange("(c p) -> p c", p=128))

        # ---------- own-batch gather r[t] = recip[idx[t]] + cumsum -> t ----------
        idxr_i32 = pp.tile([1, T], I32, tag="idxr_i32")
        nc.sync.dma_start(idxr_i32[:], tok_own.rearrange("(a t) -> a t", a=1))
        nc.vector.tensor_copy(row(R_IDX), idxr_i32[:])

        idx_bcast = pp.tile([128, T], F32, tag="idx_bcast")
        for hf in range(2):
            ib_ps = ps.tile([128, 512], F32, tag="att", bufs=4, name="ib_ps")
            nc.tensor.matmul(ib_ps[:], ones_rows[0:1, :],
                             R_IDX[0:1, hf * 512 : (hf + 1) * 512],
                             start=True, stop=True)
            nc.vector.tensor_copy(idx_bcast[:, hf * 512 : (hf + 1) * 512], ib_ps[:])

        r_ps = [ps.tile([1, 512], F32, tag="att", bufs=4, name="r_ps")
                for _ in range(2)]
        for j in range(8):
            oht = ohp.tile([128, 1024], F32, tag="oht")
            nc.vector.tensor_scalar(oht[:], idx_bcast[:], bin_cols[:, j : j + 1],
                                    None, ALU.is_equal)
            for hf in range(2):
                nc.tensor.matmul(r_ps[hf][:], recip_cols[:, j : j + 1],
                                 oht[:, hf * 512 : (hf + 1) * 512],
                                 start=(j == 0), stop=(j == 7))
        for hf in range(2):
            nc.vector.tensor_copy(row(R_R)[:, hf * 512 : (hf + 1) * 512], r_ps[hf][:])
        nc.vector.tensor_tensor_scan(row(R_T), row(R_R), row(R_Z), 0.0,
                                     ALU.add, ALU.add)

        # ---------- freqs + sin/cos (fp32, range-reduced) ----------
        freqs = pp.tile([128, T], F32, tag="freqs")
        for hf in range(2):
            tb_ps = ps.tile([128, 512], F32, tag="att", bufs=4, name="tb_ps")
            nc.tensor.matmul(tb_ps[:], ones_rows[0:1, :],
                             R_T[0:1, hf * 512 : (hf + 1) * 512],
                             start=True, stop=True)
            nc.vector.tensor_scalar(freqs[:, hf * 512 : (hf + 1) * 512], tb_ps[:],
                                    col(C_INVF), None, ALU.mult)
        # f_red = freqs - 2*pi*int(freqs/(2*pi))  (freqs >= 0)
        scr_a = pp.tile([128, T], F32, tag="scr_a")
        scr_i = pp.tile([128, T], I32, tag="scr_i")
        scr_b = pp.tile([128, T], F32, tag="scr_b")
        scr_c = pp.tile([128, T], F32, tag="scr_c")
        nc.vector.tensor_scalar(scr_a[:], freqs[:], 1.0 / TWO_PI, None, ALU.mult)
        nc.vector.tensor_copy(scr_i[:], scr_a[:])
        nc.vector.tensor_copy(scr_b[:], scr_i[:])
        nc.vector.scalar_tensor_tensor(scr_c[:], scr_b[:], -TWO_PI, freqs[:],
                                       ALU.mult, ALU.add)          # f_red
        nc.vector.add_range_wrap(scr_a[:], scr_c[:], 0.0, math.pi, TWO_PI)
        sin_f32 = pp.tile([128, T], F32, tag="sin_f32")
        nc.scalar.activation(sin_f32[:], scr_a[:], ACTF.Sin)
        nc.vector.add_range_wrap(scr_b[:], scr_c[:], math.pi / 2.0, math.pi, TWO_PI)
        cos_f32 = pp.tile([128, T], F32, tag="cos_f32")
        nc.scalar.activation(cos_f32[:], scr_b[:], ACTF.Sin)
        nc.vector.tensor_copy(cos_bf[:], cos_f32[:])
        # sin rows 0:64 negated (rotate_half sign folded in)
        nc.vector.tensor_scalar(sin_bf[0:64, :], sin_f32[0:64, :], -1.0,
                                None, ALU.mult)
        nc.vector.tensor_copy(sin_bf[64:128, :], sin_f32[64:128, :])
        if dbg is not None:
            for i, r in enumerate([R_CNT, R_RCP, R_IDX, R_R, R_T]):
                nc.sync.dma_start(dbg["dbg_rows"][i : i + 1, :], r[:])
            nc.sync.dma_start(dbg["dbg_trig"][0], scr_c[:])   # f_red
            nc.sync.dma_start(dbg["dbg_trig"][1], scr_a[:])   # sin_in
            nc.sync.dma_start(dbg["dbg_trig"][2], scr_b[:])   # cos_in
            nc.sync.dma_start(dbg["dbg_trig"][3], sin_f32[:])
            nc.sync.dma_start(dbg["dbg_trig"][4], cos_f32[:])


def _body(nc, tc, xT, w_qk, w_v, w_p, tok_all, tok_own, cs, pad, out, dbg=None):
    from contextlib import ExitStack

    with ExitStack() as ctx:
        persist = ctx.enter_context(tc.tile_pool(name="persist", bufs=1))
        ps = ctx.enter_context(tc.tile_pool(name="ps", bufs=3, space="PSUM"))
        pt_pool = ctx.enter_context(tc.tile_pool(name="pt", bufs=12))

        # ---------- persistent tiles ----------
        q_all = persist.tile([128, NHC * T], BF16, tag="q_all")
        k_all = persist.tile([128, NHC * T], BF16, tag="k_all")
        v_all = persist.tile([128, QTILES * NHC * HD], BF16, tag="v_all")
        y_big = persist.tile([128, NHC * T], BF16, tag="y_big")
        cos_bf = persist.tile([128, T], BF16, tag="cos_bf")
        sin_bf = persist.tile([128, T], BF16, tag="sin_bf")
        ones_sq = persist.tile([128, 128], BF16, tag="ones_sq")
        evs_cols = persist.tile([128, QTILES], F32, tag="evs_cols")
        klr_bf = persist.tile([1, T], BF16, tag="klr_bf")
        nc.vector.memset(ones_sq[:], 1.0)

        # ---------- positions / trig / small rows (scoped; SBUF released) ----
        _positions(nc, tc, persist, ps, tok_all, tok_own, cs, pad,
                   cos_bf, sin_bf, evs_cols, klr_bf, dbg)

        # ========== QKV + RoPE + attention, pipelined per head ==========
        with tc.tile_pool(name="bigin", bufs=1) as bigin, \
             tc.tile_pool(name="wstream", bufs=4) as wst, \
             tc.tile_pool(name="rope", bufs=3) as rope_pool, \
             tc.tile_pool(name="attp", bufs=3) as attp:
            xT_sb = bigin.tile([128, KT * T], BF16, tag="xT_sb")
            for k in range(KT):
                nc.sync.dma_start(xT_sb[:, k * T : (k + 1) * T],
                                  xT[k * 128 : (k + 1) * 128, :])
            w_v_sb = bigin.tile([128, KT * NHC * HD], BF16, tag="w_v_sb")
            for k in range(KT):
                nc.sync.dma_start(w_v_sb[:, k * 1024 : (k + 1) * 1024],
                                  w_v[k * 128 : (k + 1) * 128, :])
            one_row = bigin.tile([1, T], BF16, tag="one_row")
            nc.vector.memset(one_row[:], 1.0)

            # v first: out(tok, vcol) = xT_tile.T @ w_v (token-major, evs-scaled)
            for mt in range(QTILES):
                vv_ps = [ps.tile([128, 512], F32, tag="mm", bufs=4, name="vv_ps")
                         for _ in range(2)]
                for k in range(KT):
                    for nc2 in range(2):
                        nc.tensor.matmul(
                            vv_ps[nc2][:],
                            xT_sb[:, k * T + mt * 128 : k * T + mt * 128 + 128],
                            w_v_sb[:, k * 1024 + nc2 * 512 : k * 1024 + nc2 * 512 + 512],
                            start=(k == 0), stop=(k == KT - 1))
                for nc2 in range(2):
                    nc.vector.tensor_scalar(
                        v_all[:, mt * 1024 + nc2 * 512 : mt * 1024 + nc2 * 512 + 512],
                        vv_ps[nc2][:], evs_cols[:, mt : mt + 1], None, ALU.mult)

            for h in range(NHC):
                # --- q,k matmuls for this head (w_qk host layout: [qh|kh] per head)
                qk_ps = [[ps.tile([128, 512], F32, tag="mm", bufs=4, name="qk_ps")
                          for _ in range(2)] for _ in range(2)]
                for k in range(KT):
                    wt = wst.tile([128, 256], BF16, tag="wt")
                    nc.sync.dma_start(wt[:], w_qk[k * 128 : (k + 1) * 128,
                                                  h * 256 : (h + 1) * 256])
                    for t2 in range(2):
                        for nc2 in range(2):
                            nc.tensor.matmul(
                                qk_ps[t2][nc2][:], wt[:, t2 * 128 : t2 * 128 + 128],
                                xT_sb[:, k * T + nc2 * 512 : k * T + nc2 * 512 + 512],
                                start=(k == 0), stop=(k == KT - 1))
                for t2, dst in ((0, q_all), (1, k_all)):
                    for nc2 in range(2):
                        nc.vector.tensor_copy(
                            dst[:, h * T + nc2 * 512 : h * T + nc2 * 512 + 512],
                            qk_ps[t2][nc2][:])

                # --- RoPE on this head
                sl = slice(h * T, (h + 1) * T)
                for t_all in (q_all, k_all):
                    rot = rope_pool.tile([128, T], BF16, tag="rot", name="rot")
                    nc.sync.dma_start(rot[0:64, :], t_all[64:128, sl])
                    nc.sync.dma_start(rot[64:128, :], t_all[0:64, sl])
                    tmp = rope_pool.tile([128, T], BF16, tag="ropetmp", name="tmp")
                    nc.vector.tensor_tensor(tmp[:], t_all[:, sl], cos_bf[:], ALU.mult)
                    nc.vector.tensor_tensor(rot[:], rot[:], sin_bf[:], ALU.mult)
                    nc.vector.tensor_tensor(t_all[:, sl], tmp[:], rot[:], ALU.add)
                # last-rotary-channel overrides (engine ops can't address
                # partition 127 -> write via DMA)
                nc.sync.dma_start(q_all[127:128, sl], one_row[:])
                nc.sync.dma_start(k_all[127:128, sl], klr_bf[:])
                if dbg is not None and h == NHC - 1:
                    nc.sync.dma_start(dbg["dbg_q"][:], q_all[:])
                    nc.sync.dma_start(dbg["dbg_k"][:], k_all[:])
                    nc.sync.dma_start(dbg["dbg_v"][:], v_all[:])

                # --- attention for this head (transposed scores)
                qh = q_all[:, sl]
                for qc in range(2):
                    ktmax = (qc + 1) * 4
                    p_tiles = []
                    for kt in range(ktmax):
                        s_ps = ps.tile([128, 512], F32, tag="att", bufs=4,
                                       name="s_ps")
                        nc.tensor.matmul(
                            s_ps[:],
                            k_all[:, h * T + kt * 128 : h * T + kt * 128 + 128],
                            qh[:, qc * 512 : qc * 512 + 512],
                            start=True, stop=True)
                        p_sb = pt_pool.tile([128, 512], BF16, tag="p", name="p_sb")
                        nc.scalar.activation(p_sb[:], s_ps[:], ACTF.Exp, scale=SCALE)
                        if qc * 4 <= kt:  # diagonal-crossing tile: zero k > q
                            nc.gpsimd.affine_select(
                                p_sb[:], p_sb[:], [[1, 512]], ALU.is_ge, 0.0,
                                base=qc * 512 - kt * 128, channel_multiplier=-1)
                        if dbg is not None and h == 0 and qc == 0 and kt == 0:
                            nc.sync.dma_start(dbg["dbg_p"][:], p_sb[:])
                        p_tiles.append(p_sb)

                    z_ps = ps.tile([128, 512], F32, tag="att", bufs=4, name="z_ps")
                    for kt in range(ktmax):
                        nc.tensor.matmul(z_ps[:], ones_sq[:], p_tiles[kt][:],
                                         start=(kt == 0), stop=(kt == ktmax - 1))
                    z_sb = attp.tile([128, 512], F32, tag="z_sb")
                    nc.vector.tensor_copy(z_sb[:], z_ps[:])
                    if dbg is not None and h == 0 and qc == 0:
                        nc.sync.dma_start(dbg["dbg_z"][:], z_sb[:])
                    rz32 = attp.tile([128, 512], F32, tag="rz32")
                    nc.vector.reciprocal_approx_fast(rz32[:], z_sb[:])
                    rzb = attp.tile([128, 512], BF16, tag="rzb")
                    nc.vector.tensor_copy(rzb[:], rz32[:])

                    y_ps = ps.tile([128, 512], F32, tag="att", bufs=4, name="y_ps")
                    for kt in range(ktmax):
                        nc.tensor.matmul(
                            y_ps[:],
                            v_all[:, kt * 1024 + h * 128 : kt * 1024 + h * 128 + 128],
                            p_tiles[kt][:], start=(kt == 0), stop=(kt == ktmax - 1))
                    y_sb = attp.tile([128, 512], BF16, tag="y_sb")
                    nc.vector.tensor_copy(y_sb[:], y_ps[:])
                    nc.vector.tensor_tensor(
                        y_big[:, h * T + qc * 512 : h * T + qc * 512 + 512],
                        y_sb[:], rzb[:], ALU.mult)

        if dbg is not None:
            nc.sync.dma_start(dbg["dbg_y"][:], y_big[:])

        # ================= output projection (partial) =================
        with tc.tile_pool(name="wpp", bufs=1) as wpp, \
             tc.tile_pool(name="outp", bufs=3) as outp:
            w_p_sb = wpp.tile([128, NHC * C], BF16, tag="w_p_sb")
            for h8 in range(NHC):
                nc.sync.dma_start(w_p_sb[:, h8 * C : (h8 + 1) * C],
                                  w_p[h8 * 128 : (h8 + 1) * 128, :])
            for qt in range(QTILES):
                o_ps4 = [ps.tile([128, 512], F32, tag="mm", bufs=4, name="o_ps")
                         for _ in range(4)]
                for h8 in range(NHC):
                    for n4 in range(4):
                        nc.tensor.matmul(
                            o_ps4[n4][:],
                            y_big[:, h8 * T + qt * 128 : h8 * T + qt * 128 + 128],
                            w_p_sb[:, h8 * C + n4 * 512 : h8 * C + n4 * 512 + 512],
                            start=(h8 == 0), stop=(h8 == NHC - 1))
                for n4 in range(4):
                    o_sb = outp.tile([128, 512], F32, tag="o_sb")
                    nc.vector.tensor_copy(o_sb[:], o_ps4[n4][:])
                    nc.sync.dma_start(
                        out[qt * 128 : (qt + 1) * 128, n4 * 512 : (n4 + 1) * 512],
                        o_sb[:])


_NC_CACHE = None


def _get_nc():
    global _NC_CACHE
    if _NC_CACHE is None:
        nc = bacc.Bacc("TRN2", target_bir_lowering=False, debug=False,
                       num_devices=NCORES)
        _emit(nc)
        nc.compile()
        _NC_CACHE = nc
    return _NC_CACHE


def make_in_maps(x, cumulative_scores, token_index, padding_mask, W_attn, W_proj):
    bf = ml_dtypes.bfloat16
    x = np.asarray(x, np.float32)
    csf = np.asarray(cumulative_scores, np.float32)
    tok = np.asarray(token_index, np.int32)
    padf = np.asarray(padding_mask, np.float32)
    Wa = np.asarray(W_attn, np.float32)
    Wp = np.asarray(W_proj, np.float32)

    tok_all = np.ascontiguousarray(tok.reshape(B * T))
    in_maps = []
    for core in range(NCORES):
        b, hg = core // 2, core % 2
        cols = slice(hg * 1024, (hg + 1) * 1024)
        wq = Wa[:, 0 * C : 1 * C][:, cols]
        wk = Wa[:, 1 * C : 2 * C][:, cols]
        wv = Wa[:, 2 * C : 3 * C][:, cols]
        in_maps.append({
            "xT": np.ascontiguousarray(x[b].T).astype(bf),
            "w_qk": np.ascontiguousarray(np.concatenate(
                [np.concatenate([wq[:, hh * 128 : (hh + 1) * 128],
                                 wk[:, hh * 128 : (hh + 1) * 128]], axis=1)
                 for hh in range(NHC)], axis=1)).astype(bf),
            "w_v": np.ascontiguousarray(wv).astype(bf),
            "w_p": np.ascontiguousarray(Wp[hg * 1024 : (hg + 1) * 1024, :]).astype(bf),
            "tok_all": tok_all,
            "tok_own": np.ascontiguousarray(tok[b]),
            "cs": np.ascontiguousarray(csf[b]),
            "pad": np.ascontiguousarray(padf[b]),
        })
    return in_maps


def kernel(x, cumulative_scores, token_index, padding_mask, W_attn, W_proj):
    nc = _get_nc()
    in_maps = make_in_maps(x, cumulative_scores, token_index, padding_mask,
                           W_attn, W_proj)
    res = run_bass_kernel_spmd(nc, in_maps, list(range(NCORES)))
    outs = [res.results[c]["out"] for c in range(NCORES)]
    full = np.stack([outs[2 * b] + outs[2 * b + 1] for b in range(B)], axis=0)
    return full.astype(np.float32)


# revision 30
# speedup vs baseline: 1.6799x; 1.2807x over previous
"""Trainium2 Bass kernel for nn_CausalSelfAttention_16003048145608.

Problem (see reference semantics): B=4, T=1024, C=2048, H=16 heads, HD=128.
  qkv = x @ W_attn ; split q,k,v ; input-dependent RoPE positions t derived
  from a histogram of token_index over ALL batch rows + per-row gather/cumsum;
  RoPE(q,k) with per-token angle t; q[...,-1]=1, k[...,-1]=cumulative_scores;
  v *= exp(cumulative_scores); causal+padding softmax(q k^T/sqrt(HD)) @ v;
  out = y @ W_proj.

Sharding (8 cores): batch(4) x head-group(2 groups of 8 heads).
  Core c handles batch b=c//2, heads hg=c%2 (8 heads).  QKV is a column slice
  of W_attn (tensor parallel), attention is per-head, proj is a row slice of
  W_proj producing a partial (1024, 2048) output; host sums the 2 partials
  per batch (the only cross-core reduction).

On-core layouts (partition dim first):
  xT      (C=2048 -> 16 k-tiles of 128, T=1024)   bf16, resident
  qT/kT   (128 = head dim d, 8 heads * 1024 tok)  bf16  ("q_all"/"k_all")
  v       (128 = token tile, 8 tok-tiles * 1024 vcol) bf16 ("v_all"),
          computed token-major directly (x as stationary operand) so the
          attention O = P^T V matmul needs no transposes at all.
  scores  computed TRANSPOSED: sT(k, q) = lhsT(kT-tile).T @ qT -> softmax
          denominators via a ones(128,128) matmul (partition reduction on PE).
All matmuls bf16 with fp32 PSUM accumulation; positions/trig in fp32.
Padding is folded into the k last-channel override (cs + (pad-1)*1e9) so the
masked columns exp() to exactly 0 with no per-element mask work.
"""

import math
import numpy as np
import ml_dtypes

import concourse.bass as bass
import concourse.bacc as bacc
import concourse.tile as tile
from concourse import mybir
from concourse.bass_utils import run_bass_kernel_spmd

F32 = mybir.dt.float32
BF16 = mybir.dt.float16  # 16-bit matmul dtype: fp16 (same PE speed as bf16, 8x mantissa)
I32 = mybir.dt.int32
ALU = mybir.AluOpType
ACTF = mybir.ActivationFunctionType

B, T, C, H, HD = 4, 1024, 2048, 16, 128
NHC = 8                 # heads per core
NCORES = 8
KT = C // 128           # 16 contraction tiles for qkv
QTILES = T // 128       # 8
SCALE = 1.0 / math.sqrt(HD)
TWO_PI = 2.0 * math.pi
LN1E4 = math.log(10000.0)


DEBUG = False
REPEAT = 1   # emit the body N times (for slope-based HW timing)


def _emit(nc):
    # ---- DRAM I/O (per-core shapes; host feeds per-core slices) ----
    xT = nc.dram_tensor("xT", [C, T], BF16, kind="ExternalInput").ap()
    w_qk = nc.dram_tensor("w_qk", [C, 2 * NHC * HD], BF16, kind="ExternalInput").ap()
    w_v = nc.dram_tensor("w_v", [C, NHC * HD], BF16, kind="ExternalInput").ap()
    w_p = nc.dram_tensor("w_p", [NHC * HD, C], BF16, kind="ExternalInput").ap()
    tok_all = nc.dram_tensor("tok_all", [B * T], I32, kind="ExternalInput").ap()
    tok_own = nc.dram_tensor("tok_own", [T], I32, kind="ExternalInput").ap()
    cs = nc.dram_tensor("cs", [T], F32, kind="ExternalInput").ap()
    pad = nc.dram_tensor("pad", [T], F32, kind="ExternalInput").ap()
    out = nc.dram_tensor("out", [T, C], F32, kind="ExternalOutput").ap()

    dbg = None
    if DEBUG:
        dbg = {
            "dbg_rows": nc.dram_tensor("dbg_rows", [8, 1024], F32,
                                       kind="ExternalOutput").ap(),
            "dbg_trig": nc.dram_tensor("dbg_trig", [5, 128, 1024], F32,
                                       kind="ExternalOutput").ap(),
            "dbg_q": nc.dram_tensor("dbg_q", [128, 8192], BF16,
                                    kind="ExternalOutput").ap(),
            "dbg_k": nc.dram_tensor("dbg_k", [128, 8192], BF16,
                                    kind="ExternalOutput").ap(),
            "dbg_v": nc.dram_tensor("dbg_v", [128, 8192], BF16,
                                    kind="ExternalOutput").ap(),
            "dbg_y": nc.dram_tensor("dbg_y", [128, 8192], BF16,
                                    kind="ExternalOutput").ap(),
            "dbg_p": nc.dram_tensor("dbg_p", [128, 512], BF16,
                                    kind="ExternalOutput").ap(),
            "dbg_z": nc.dram_tensor("dbg_z", [128, 512], F32,
                                    kind="ExternalOutput").ap(),
        }

    with tile.TileContext(nc) as tc:
        for _ in range(REPEAT):
            _body(nc, tc, xT, w_qk, w_v, w_p, tok_all, tok_own, cs, pad, out, dbg)
    return nc


def _positions(nc, tc, persist, ps, tok_all, tok_own, cs, pad,
               cos_bf, sin_bf, evs_cols, klr_bf, dbg=None):
    """Input-dependent RoPE positions -> cos_bf / sin_bf (sin rows 0:64 are
    negated), plus evs_cols = exp(cs) columns and the k-last-channel row."""
    with tc.tile_pool(name="posp", bufs=1) as pp, \
         tc.tile_pool(name="ohp", bufs=3) as ohp:
        # scalar rows packed into partitions of one tile (1 row each)
        # scalar rows: the HW verifier requires TensorTensor/STT/scan SBUF
        # operands to share the SAME start partition, so every row lives at
        # partition 0 of its own tile.
        def mkrow(tag):
            r = pp.tile([1, 1024], F32, tag=tag, name=tag)
            return r

        R_CS, R_PAD, R_KLT = mkrow("r_cs"), mkrow("r_pad"), mkrow("r_klt")
        R_CNT, R_RCP, R_IDX = mkrow("r_cnt"), mkrow("r_rcp"), mkrow("r_idx")
        R_R, R_Z, R_T = mkrow("r_r"), mkrow("r_z"), mkrow("r_t")
        nc.vector.memset(R_Z[:], 0.0)

        def row(r):
            return r[:]

        cs_cols = pp.tile([128, QTILES], F32, tag="cs_cols")
        nc.sync.dma_start(cs_cols[:], cs.rearrange("(c p) -> p c", p=128))
        nc.scalar.activation(evs_cols[:], cs_cols[:], ACTF.Exp)

        nc.sync.dma_start(row(R_CS), cs.rearrange("(a t) -> a t", a=1))
        nc.sync.dma_start(row(R_PAD), pad.rearrange("(a t) -> a t", a=1))
        # k-last-channel override row: cs + (pad-1)*1e9  (padding -> -1e9)
        nc.vector.tensor_scalar(row(R_KLT), row(R_PAD), 6e4, -6e4, ALU.mult, ALU.add)
        nc.vector.tensor_tensor(klr_bf[:], row(R_KLT), row(R_CS), ALU.add)

        # ---------- iotas ----------
        iota_i32 = pp.tile([128, 1024], I32, tag="iota_i32")
        nc.gpsimd.iota(iota_i32[:], [[1, 1024]], base=0, channel_multiplier=0)
        iota_f32 = pp.tile([128, 1024], F32, tag="iota_f32")
        nc.vector.tensor_copy(iota_f32[:], iota_i32[:])

        cols = pp.tile([128, 16], F32, tag="cols")  # small per-partition columns
        C_IOTA, C_GE, C_RM, C_EARG, C_INVF = range(5)

        def col(i):
            return cols[:, i : i + 1]

        iotap_i32 = pp.tile([128, 1], I32, tag="iotap_i32")
        nc.gpsimd.iota(iotap_i32[:], [[1, 1]], base=0, channel_multiplier=1)
        nc.vector.tensor_copy(col(C_IOTA), iotap_i32[:])
        bin_cols = pp.tile([128, 8], F32, tag="bin_cols")
        for j in range(8):
            nc.vector.tensor_scalar(bin_cols[:, j : j + 1], col(C_IOTA),
                                    float(128 * j), None, ALU.add)

        # inv_freq column (128,1): rows r -> 10000^(-2*(r%64)/128)
        nc.vector.tensor_scalar(col(C_GE), col(C_IOTA), 64.0, None, ALU.is_ge)
        nc.vector.scalar_tensor_tensor(col(C_RM), col(C_GE), -64.0, col(C_IOTA),
                                       ALU.mult, ALU.add)
        nc.vector.tensor_scalar(col(C_EARG), col(C_RM), -2.0 * LN1E4 / 128.0,
                                None, ALU.mult)
        nc.scalar.activation(col(C_INVF), col(C_EARG), ACTF.Exp)

        # ---------- histogram of tok_all (counts over ALL batch rows) ----------
        tok_cols_i32 = pp.tile([128, 32], I32, tag="tok_cols_i32")
        nc.sync.dma_start(tok_cols_i32[:], tok_all.rearrange("(c p) -> p c", p=128))
        tok_cols = pp.tile([128, 32], F32, tag="tok_cols")
        nc.vector.tensor_copy(tok_cols[:], tok_cols_i32[:])
        ones_col = pp.tile([128, 1], BF16, tag="ones_col")
        nc.vector.memset(ones_col[:], 1.0)
        ones_rows = pp.tile([128, 128], F32, tag="ones_rows")
        nc.vector.memset(ones_rows[:], 1.0)

        counts_ps = [ps.tile([1, 512], F32, tag="att", bufs=4, name="counts_ps")
                     for _ in range(2)]
        for c in range(32):
            oh = ohp.tile([128, 1024], BF16, tag="oh")
            nc.vector.tensor_scalar(oh[:], iota_f32[:], tok_cols[:, c : c + 1],
                                    None, ALU.is_equal)
            for hf in range(2):
                nc.tensor.matmul(counts_ps[hf][:], ones_col[:],
                                 oh[:, hf * 512 : (hf + 1) * 512],
                                 start=(c == 0), stop=(c == 31))
        for hf in range(2):
            nc.vector.tensor_scalar(row(R_CNT)[:, hf * 512 : (hf + 1) * 512],
                                    counts_ps[hf][:], 1e-10, None, ALU.add)
        nc.vector.reciprocal_approx_fast(row(R_RCP), row(R_CNT))
        # zero-count bins are never gathered; clamp their huge recips to 0 so
        # any numerical dust in the gather matmul cannot be amplified by 1e10
        R_M = mkrow("r_m")
        nc.vector.tensor_scalar(row(R_M), row(R_CNT), 0.5, None, ALU.is_ge)
        R_RCPC = mkrow("r_rcpc")
        nc.vector.tensor_tensor(row(R_RCPC), row(R_RCP), row(R_M), ALU.mult)
        # row -> column layout via a DRAM bounce (SBUF->SBUF transposing DMA
        # miscopies on HW; DRAM->SBUF partition-major loads are the proven path)
        rcp_dram = nc.dram_tensor(f"scratch_rcp_{nc.next_id()}", [1024], F32).ap()
        nc.sync.dma_start(rcp_dram.rearrange("(a t) -> a t", a=1), row(R_RCPC))
        recip_cols = pp.tile([128, 8], F32, tag="recip_cols")
        nc.sync.dma_start(recip_cols[:], rcp_dram.rearrange("(c p) -> p c", p=128))

        # ---------- own-batch gather r[t] = recip[idx[t]] + cumsum -> t ----------
        idxr_i32 = pp.tile([1, T], I32, tag="idxr_i32")
        nc.sync.dma_start(idxr_i32[:], tok_own.rearrange("(a t) -> a t", a=1))
        nc.vector.tensor_copy(row(R_IDX), idxr_i32[:])

        idx_bcast = pp.tile([128, T], F32, tag="idx_bcast")
        for hf in range(2):
            ib_ps = ps.tile([128, 512], F32, tag="att", bufs=4, name="ib_ps")
            nc.tensor.matmul(ib_ps[:], ones_rows[0:1, :],
                             R_IDX[0:1, hf * 512 : (hf + 1) * 512],
                             start=True, stop=True)
            nc.vector.tensor_copy(idx_bcast[:, hf * 512 : (hf + 1) * 512], ib_ps[:])

        r_ps = [ps.tile([1, 512], F32, tag="att", bufs=4, name="r_ps")
                for _ in range(2)]
        for j in range(8):
            oht = ohp.tile([128, 1024], F32, tag="oht")
            nc.vector.tensor_scalar(oht[:], idx_bcast[:], bin_cols[:, j : j + 1],
                                    None, ALU.is_equal)
            for hf in range(2):
                nc.tensor.matmul(r_ps[hf][:], recip_cols[:, j : j + 1],
                                 oht[:, hf * 512 : (hf + 1) * 512],
                                 start=(j == 0), stop=(j == 7))
        for hf in range(2):
            nc.vector.tensor_copy(row(R_R)[:, hf * 512 : (hf + 1) * 512], r_ps[hf][:])
        nc.vector.tensor_tensor_scan(row(R_T), row(R_R), row(R_Z), 0.0,
                                     ALU.add, ALU.add)

        # ---------- freqs + sin/cos (fp32, range-reduced) ----------
        freqs = pp.tile([128, T], F32, tag="freqs")
        for hf in range(2):
            tb_ps = ps.tile([128, 512], F32, tag="att", bufs=4, name="tb_ps")
            nc.tensor.matmul(tb_ps[:], ones_rows[0:1, :],
                             R_T[0:1, hf * 512 : (hf + 1) * 512],
                             start=True, stop=True)
            nc.vector.tensor_scalar(freqs[:, hf * 512 : (hf + 1) * 512], tb_ps[:],
                                    col(C_INVF), None, ALU.mult)
        # f_red = freqs - 2*pi*int(freqs/(2*pi))  (freqs >= 0)
        scr_a = pp.tile([128, T], F32, tag="scr_a")
        scr_i = pp.tile([128, T], I32, tag="scr_i")
        scr_b = pp.tile([128, T], F32, tag="scr_b")
        scr_c = pp.tile([128, T], F32, tag="scr_c")
        nc.vector.tensor_scalar(scr_a[:], freqs[:], 1.0 / TWO_PI, None, ALU.mult)
        nc.vector.tensor_copy(scr_i[:], scr_a[:])
        nc.vector.tensor_copy(scr_b[:], scr_i[:])
        nc.vector.scalar_tensor_tensor(scr_c[:], scr_b[:], -TWO_PI, freqs[:],
                                       ALU.mult, ALU.add)          # f_red
        nc.vector.add_range_wrap(scr_a[:], scr_c[:], 0.0, math.pi, TWO_PI)
        sin_f32 = pp.tile([128, T], F32, tag="sin_f32")
        nc.scalar.activation(sin_f32[:], scr_a[:], ACTF.Sin)
        nc.vector.add_range_wrap(scr_b[:], scr_c[:], math.pi / 2.0, math.pi, TWO_PI)
        cos_f32 = pp.tile([128, T], F32, tag="cos_f32")
        nc.scalar.activation(cos_f32[:], scr_b[:], ACTF.Sin)
        nc.vector.tensor_copy(cos_bf[:], cos_f32[:])
        # sin rows 0:64 negated (rotate_half sign folded in)
        nc.vector.tensor_scalar(sin_bf[0:64, :], sin_f32[0:64, :], -1.0,
                                None, ALU.mult)
        nc.vector.tensor_copy(sin_bf[64:128, :], sin_f32[64:128, :])
        if dbg is not None:
            for i, r in enumerate([R_CNT, R_RCP, R_IDX, R_R, R_T]):
                nc.sync.dma_start(dbg["dbg_rows"][i : i + 1, :], r[:])
            nc.sync.dma_start(dbg["dbg_trig"][0], scr_c[:])   # f_red
            nc.sync.dma_start(dbg["dbg_trig"][1], scr_a[:])   # sin_in
            nc.sync.dma_start(dbg["dbg_trig"][2], scr_b[:])   # cos_in
            nc.sync.dma_start(dbg["dbg_trig"][3], sin_f32[:])
            nc.sync.dma_start(dbg["dbg_trig"][4], cos_f32[:])


def _body(nc, tc, xT, w_qk, w_v, w_p, tok_all, tok_own, cs, pad, out, dbg=None):
    from contextlib import ExitStack

    with ExitStack() as ctx:
        persist = ctx.enter_context(tc.tile_pool(name="persist", bufs=1))
        ps = ctx.enter_context(tc.tile_pool(name="ps", bufs=3, space="PSUM"))
        pt_pool = ctx.enter_context(tc.tile_pool(name="pt", bufs=12))

        # ---------- persistent tiles ----------
        q_all = persist.tile([128, NHC * T], BF16, tag="q_all")
        k_all = persist.tile([128, NHC * T], BF16, tag="k_all")
        v_all = persist.tile([128, QTILES * NHC * HD], BF16, tag="v_all")
        y_big = persist.tile([128, NHC * T], BF16, tag="y_big")
        cos_bf = persist.tile([128, T], BF16, tag="cos_bf")
        sin_bf = persist.tile([128, T], BF16, tag="sin_bf")
        ones_sq = persist.tile([128, 128], BF16, tag="ones_sq")
        evs_cols = persist.tile([128, QTILES], F32, tag="evs_cols")
        klr_bf = persist.tile([1, T], BF16, tag="klr_bf")
        nc.vector.memset(ones_sq[:], 1.0)

        # ---------- positions / trig / small rows (scoped; SBUF released) ----
        _positions(nc, tc, persist, ps, tok_all, tok_own, cs, pad,
                   cos_bf, sin_bf, evs_cols, klr_bf, dbg)

        # ========== QKV + RoPE + attention, pipelined per head ==========
        with tc.tile_pool(name="bigin", bufs=1) as bigin, \
             tc.tile_pool(name="wstream", bufs=4) as wst, \
             tc.tile_pool(name="rope", bufs=3) as rope_pool, \
             tc.tile_pool(name="attp", bufs=3) as attp:
            xT_sb = bigin.tile([128, KT * T], BF16, tag="xT_sb")
            for k in range(KT):
                nc.sync.dma_start(xT_sb[:, k * T : (k + 1) * T],
                                  xT[k * 128 : (k + 1) * 128, :])
            w_v_sb = bigin.tile([128, KT * NHC * HD], BF16, tag="w_v_sb")
            for k in range(KT):
                nc.sync.dma_start(w_v_sb[:, k * 1024 : (k + 1) * 1024],
                                  w_v[k * 128 : (k + 1) * 128, :])
            one_row = bigin.tile([1, T], BF16, tag="one_row")
            nc.vector.memset(one_row[:], 1.0)

            # v first: out(tok, vcol) = xT_tile.T @ w_v (token-major, evs-scaled)
            for mt in range(QTILES):
                vv_ps = [ps.tile([128, 512], F32, tag="mm", bufs=4, name="vv_ps")
                         for _ in range(2)]
                for k in range(KT):
                    for nc2 in range(2):
                        nc.tensor.matmul(
                            vv_ps[nc2][:],
                            xT_sb[:, k * T + mt * 128 : k * T + mt * 128 + 128],
                            w_v_sb[:, k * 1024 + nc2 * 512 : k * 1024 + nc2 * 512 + 512],
                            start=(k == 0), stop=(k == KT - 1))
                for nc2 in range(2):
                    nc.vector.tensor_scalar(
                        v_all[:, mt * 1024 + nc2 * 512 : mt * 1024 + nc2 * 512 + 512],
                        vv_ps[nc2][:], evs_cols[:, mt : mt + 1], None, ALU.mult)

            for h in range(NHC):
                # --- q,k matmuls for this head (w_qk host layout: [qh|kh] per head)
                qk_ps = [[ps.tile([128, 512], F32, tag="mm", bufs=4, name="qk_ps")
                          for _ in range(2)] for _ in range(2)]
                for k in range(KT):
                    wt = wst.tile([128, 256], BF16, tag="wt")
                    nc.sync.dma_start(wt[:], w_qk[k * 128 : (k + 1) * 128,
                                                  h * 256 : (h + 1) * 256])
                    for t2 in range(2):
                        for nc2 in range(2):
                            nc.tensor.matmul(
                                qk_ps[t2][nc2][:], wt[:, t2 * 128 : t2 * 128 + 128],
                                xT_sb[:, k * T + nc2 * 512 : k * T + nc2 * 512 + 512],
                                start=(k == 0), stop=(k == KT - 1))
                for t2, dst in ((0, q_all), (1, k_all)):
                    for nc2 in range(2):
                        nc.vector.tensor_copy(
                            dst[:, h * T + nc2 * 512 : h * T + nc2 * 512 + 512],
                            qk_ps[t2][nc2][:])

                # --- RoPE on this head
                sl = slice(h * T, (h + 1) * T)
                for t_all in (q_all, k_all):
                    rot = rope_pool.tile([128, T], BF16, tag="rot", name="rot")
                    nc.sync.dma_start(rot[0:64, :], t_all[64:128, sl])
                    nc.sync.dma_start(rot[64:128, :], t_all[0:64, sl])
                    tmp = rope_pool.tile([128, T], BF16, tag="ropetmp", name="tmp")
                    nc.vector.tensor_tensor(tmp[:], t_all[:, sl], cos_bf[:], ALU.mult)
                    nc.vector.tensor_tensor(rot[:], rot[:], sin_bf[:], ALU.mult)
                    nc.vector.tensor_tensor(t_all[:, sl], tmp[:], rot[:], ALU.add)
                # last-rotary-channel overrides (engine ops can't address
                # partition 127 -> write via DMA)
                nc.sync.dma_start(q_all[127:128, sl], one_row[:])
                nc.sync.dma_start(k_all[127:128, sl], klr_bf[:])
                if dbg is not None and h == NHC - 1:
                    nc.sync.dma_start(dbg["dbg_q"][:], q_all[:])
                    nc.sync.dma_start(dbg["dbg_k"][:], k_all[:])
                    nc.sync.dma_start(dbg["dbg_v"][:], v_all[:])

                # --- attention for this head (transposed scores)
                qh = q_all[:, sl]
                for qc in range(2):
                    ktmax = (qc + 1) * 4
                    p_tiles = []
                    for kt in range(ktmax):
                        s_ps = ps.tile([128, 512], F32, tag="att", bufs=4,
                                       name="s_ps")
                        nc.tensor.matmul(
                            s_ps[:],
                            k_all[:, h * T + kt * 128 : h * T + kt * 128 + 128],
                            qh[:, qc * 512 : qc * 512 + 512],
                            start=True, stop=True)
                        p_sb = pt_pool.tile([128, 512], BF16, tag="p", name="p_sb")
                        nc.scalar.activation(p_sb[:], s_ps[:], ACTF.Exp, scale=SCALE)
                        if qc * 4 <= kt:  # diagonal-crossing tile: zero k > q
                            nc.gpsimd.affine_select(
                                p_sb[:], p_sb[:], [[1, 512]], ALU.is_ge, 0.0,
                                base=qc * 512 - kt * 128, channel_multiplier=-1)
                        if dbg is not None and h == 0 and qc == 0 and kt == 0:
                            nc.sync.dma_start(dbg["dbg_p"][:], p_sb[:])
                        p_tiles.append(p_sb)

                    # partition-sum once: elementwise-sum the P tiles on
                    # DVE (fp16 2x), then a single ones-matmul per (h, qc)
                    s_acc = attp.tile([128, 512], BF16, tag="s_acc")
                    nc.vector.tensor_tensor(s_acc[:], p_tiles[0][:],
                                            p_tiles[1][:], ALU.add)
                    for kt in range(2, ktmax):
                        nc.vector.tensor_tensor(s_acc[:], s_acc[:],
                                                p_tiles[kt][:], ALU.add)
                    z_ps = ps.tile([128, 512], F32, tag="att", bufs=4, name="z_ps")
                    nc.tensor.matmul(z_ps[:], ones_sq[:], s_acc[:],
                                     start=True, stop=True)
                    z_sb = attp.tile([128, 512], F32, tag="z_sb")
                    nc.vector.tensor_copy(z_sb[:], z_ps[:])
                    if dbg is not None and h == 0 and qc == 0:
                        nc.sync.dma_start(dbg["dbg_z"][:], z_sb[:])
                    rz32 = attp.tile([128, 512], F32, tag="rz32")
                    nc.vector.reciprocal_approx_fast(rz32[:], z_sb[:])

                    y_ps = ps.tile([128, 512], F32, tag="att", bufs=4, name="y_ps")
                    for kt in range(ktmax):
                        nc.tensor.matmul(
                            y_ps[:],
                            v_all[:, kt * 1024 + h * 128 : kt * 1024 + h * 128 + 128],
                            p_tiles[kt][:], start=(kt == 0), stop=(kt == ktmax - 1))
                    y_sb = attp.tile([128, 512], F32, tag="y_sb")
                    nc.vector.tensor_copy(y_sb[:], y_ps[:])
                    nc.vector.tensor_tensor(
                        y_big[:, h * T + qc * 512 : h * T + qc * 512 + 512],
                        y_sb[:], rz32[:], ALU.mult)

        if dbg is not None:
            nc.sync.dma_start(dbg["dbg_y"][:], y_big[:])

        # ================= output projection (partial) =================
        with tc.tile_pool(name="wpp", bufs=1) as wpp, \
             tc.tile_pool(name="outp", bufs=3) as outp:
            w_p_sb = wpp.tile([128, NHC * C], BF16, tag="w_p_sb")
            for h8 in range(NHC):
                nc.sync.dma_start(w_p_sb[:, h8 * C : (h8 + 1) * C],
                                  w_p[h8 * 128 : (h8 + 1) * 128, :])
            for qt in range(QTILES):
                o_ps4 = [ps.tile([128, 512], F32, tag="mm", bufs=4, name="o_ps")
                         for _ in range(4)]
                for h8 in range(NHC):
                    for n4 in range(4):
                        nc.tensor.matmul(
                            o_ps4[n4][:],
                            y_big[:, h8 * T + qt * 128 : h8 * T + qt * 128 + 128],
                            w_p_sb[:, h8 * C + n4 * 512 : h8 * C + n4 * 512 + 512],
                            start=(h8 == 0), stop=(h8 == NHC - 1))
                for n4 in range(4):
                    o_sb = outp.tile([128, 512], F32, tag="o_sb")
                    nc.vector.tensor_copy(o_sb[:], o_ps4[n4][:])
                    nc.sync.dma_start(
                        out[qt * 128 : (qt + 1) * 128, n4 * 512 : (n4 + 1) * 512],
                        o_sb[:])


_NC_CACHE = None


def _get_nc():
    global _NC_CACHE
    if _NC_CACHE is None:
        nc = bacc.Bacc("TRN2", target_bir_lowering=False, debug=False,
                       num_devices=NCORES)
        _emit(nc)
        nc.compile()
        _NC_CACHE = nc
    return _NC_CACHE


def make_in_maps(x, cumulative_scores, token_index, padding_mask, W_attn, W_proj):
    bf = np.float16
    x = np.asarray(x, np.float32)
    csf = np.asarray(cumulative_scores, np.float32)
    tok = np.asarray(token_index, np.int32)
    padf = np.asarray(padding_mask, np.float32)
    Wa = np.asarray(W_attn, np.float32)
    Wp = np.asarray(W_proj, np.float32)

    tok_all = np.ascontiguousarray(tok.reshape(B * T))
    in_maps = []
    for core in range(NCORES):
        b, hg = core // 2, core % 2
        cols = slice(hg * 1024, (hg + 1) * 1024)
        wq = Wa[:, 0 * C : 1 * C][:, cols]
        wk = Wa[:, 1 * C : 2 * C][:, cols]
        wv = Wa[:, 2 * C : 3 * C][:, cols]
        in_maps.append({
            "xT": np.ascontiguousarray(x[b].T).astype(bf),
            "w_qk": np.ascontiguousarray(np.concatenate(
                [np.concatenate([wq[:, hh * 128 : (hh + 1) * 128],
                                 wk[:, hh * 128 : (hh + 1) * 128]], axis=1)
                 for hh in range(NHC)], axis=1)).astype(bf),
            "w_v": np.ascontiguousarray(wv).astype(bf),
            "w_p": np.ascontiguousarray(Wp[hg * 1024 : (hg + 1) * 1024, :]).astype(bf),
            "tok_all": tok_all,
            "tok_own": np.ascontiguousarray(tok[b]),
            "cs": np.ascontiguousarray(csf[b]),
            "pad": np.ascontiguousarray(padf[b]),
        })
    return in_maps


def kernel(x, cumulative_scores, token_index, padding_mask, W_attn, W_proj):
    nc = _get_nc()
    in_maps = make_in_maps(x, cumulative_scores, token_index, padding_mask,
                           W_attn, W_proj)
    res = run_bass_kernel_spmd(nc, in_maps, list(range(NCORES)))
    outs = [res.results[c]["out"] for c in range(NCORES)]
    full = np.stack([outs[2 * b] + outs[2 * b + 1] for b in range(B)], axis=0)
    return full.astype(np.float32)
